# revision 37
# baseline (speedup 1.0000x reference)
"""Causal self-attention (B=2, L=2048, C=1024, 16 heads) on 8 Trainium2
NeuronCores via Bass/Tile.

Sharding (8 cores = 2 batches x 4 head-groups of 4 heads):
  core c: batch b = c // 4, head-group g = c % 4  (heads 4g..4g+3)

Host prep per core: x[b].T, column slices of Wq/Wk/Wv (transposed), an
output-channel slice of Wp (transposed), an additive key-mask derived from
attn_mask, and a 128x128 upper-triangular causal mask tile.  Matmul operands
are cast to bf16 on the host; accumulation is fp32 on-device (measured
output relative error vs the fp32 reference: ~4.3e-3; an fp32r build was
validated at 2.7e-4 but runs the PE at 1/4 rate).

Device program (SPMD — identical program on all cores, per-core data):
  - v = x @ Wv_slice.T in natural [token, dim] layout with a ones column per
    head appended, so the P@V matmul also produces the softmax denominator.
  - q^T,k^T [256, L]: two tiles, 2 heads each at partition offsets 0/64, via
    matmuls contracting C on the partition axis (x arrives pre-transposed).
  - attention per head-pair, per 512-query chunk, per 128-key block:
    s^T = k^T.T @ q^T (keys on partitions, queries free) -> exp on ScalarE
    with scale=1/8 and per-key-partition bias=attn_mask fused into the
    activation (scores are O(3), so no max-subtraction is needed) -> P^T in
    bf16; causal masking = memset of fully-masked query columns plus one
    128x128 triangular multiply on the diagonal block; the two heads of a
    pair use PE row groups 0-63/64-127 so their K=64 score matmuls run
    concurrently.  y^T_aug [65, chunk] = v_aug.T @ P^T accumulates over key
    blocks in PSUM; rows 0-63 are divided by row 64 (the denominator) via
    reciprocal + a K=1 ones-matmul that broadcasts 1/den into the unused
    upper rows of the same PSUM bank (PE does the broadcast so the Pool
    engine stays free for collectives).
  - per query chunk: partial projection FIRST — out_part[512, 1024] =
    y_mine^T.T @ Wp.T[my 256 rows, :] (K=256) in bf16 — then a 4-rank
    ReduceScatter(add) within the batch group sums the 4 partials and
    hands each core its own 128-query slice (256KB out vs the 1MB an
    AllGather would move, and no PE work waits on the collective).
Host assembly: interleave each core's 4x128 query rows per batch.
"""
import sys
import numpy as np
import ml_dtypes

for _p in ("/opt/trn_rl_repo",):
    if _p not in sys.path:
        sys.path.insert(0, _p)

import concourse.bass as bass
import concourse.mybir as mybir
import concourse.tile as tile
from concourse import bacc
from concourse import bass_utils

F32 = mybir.dt.float32
BF16 = mybir.dt.bfloat16
AF = mybir.ActivationFunctionType

N_CORES = 8
B, L, C, H, D = 2, 2048, 1024, 16, 64
H_PER_CORE = 4
DQ = H_PER_CORE * D          # 256 = per-core q/k/v width and out-column slice
CT = C // 128                # contraction tiles
TT = L // 128                # token tiles
QCHUNK = 512
QC = L // QCHUNK
NS = QCHUNK // 512           # 512-wide sub-chunks per query chunk
NB = L // 128                # key blocks
NEG = -30000.0


def build_kernel(use_collective=True, reps=1):
    nc = bacc.Bacc("TRN2", target_bir_lowering=False, debug=False,
                   num_devices=N_CORES)

    xT_d = nc.dram_tensor("xT", [C, L], BF16, kind="ExternalInput")
    wqkvT_d = nc.dram_tensor("wqkvT", [C, 3 * DQ], BF16,
                             kind="ExternalInput")
    wpT_d = nc.dram_tensor("wpT", [DQ, C], BF16, kind="ExternalInput")
    kmask_d = nc.dram_tensor("kmask", [128, NB], F32, kind="ExternalInput")
    trimask_d = nc.dram_tensor("trimask", [128, 128], BF16, kind="ExternalInput")
    # one bf16 output tile per query chunk, written directly by the
    # ReduceScatter (host converts to f32)
    out_ds = [nc.dram_tensor(f"out{q}", [128, C], BF16, kind="ExternalOutput")
              for q in range(QC)]

    with tile.TileContext(nc) as tc:
        import contextlib
        with contextlib.ExitStack() as ctx:
            const = ctx.enter_context(tc.tile_pool(name="const", bufs=1))
            kmask = const.tile([128, NB], F32)
            trimask = const.tile([128, 128], BF16)
            ones1 = const.tile([1, 64], BF16)

            w_pool = ctx.enter_context(tc.tile_pool(name="w", bufs=1))
            sb = ctx.enter_context(tc.tile_pool(name="sb", bufs=1))
            pt_pool = ctx.enter_context(tc.tile_pool(name="pt", bufs=6))
            den_pool = ctx.enter_context(tc.tile_pool(name="den", bufs=3))
            stg = ctx.enter_context(tc.tile_pool(name="stg", bufs=3))
            ps = ctx.enter_context(tc.tile_pool(name="ps", bufs=1, space="PSUM"))

            wqkv = [w_pool.tile([128, 3 * DQ], BF16, tag=f"wqkv{k}",
                                name=f"wqkv{k}") for k in range(CT)]
            wq = [t[:, 0:DQ] for t in wqkv]
            wk = [t[:, DQ:2*DQ] for t in wqkv]
            wv = [t[:, 2*DQ:3*DQ] for t in wqkv]
            wp = [w_pool.tile([128, C], BF16, tag=f"wp{k}", name=f"wp{k}")
                  for k in range(2)]

            def load_inputs():
                # interleave x and weight tiles (x on SP, weights on the
                # Activation queue) so the first v_proj matmuls start early
                nc.vector.memset(ones1[:], 1.0)
                # x lands column-chunk-major across three DMA queues so the
                # first v_proj/qk_proj token tiles start after ~1/4 of x
                engs = [nc.sync, nc.gpsimd, nc.scalar]
                for k in range(CT):
                    engs[k % len(engs)].dma_start(
                        out=wqkv[k][:], in_=wqkvT_d[k*128:(k+1)*128, :])
                for c in range(4):
                    for k in range(CT):
                        engs[k % len(engs)].dma_start(
                            out=xT[k][:, c*512:(c+1)*512],
                            in_=xT_d[k*128:(k+1)*128, c*512:(c+1)*512])
                for k in range(2):
                    nc.scalar.dma_start(out=wp[k][:],
                                        in_=wpT_d[k*128:(k+1)*128, :])
                nc.scalar.dma_start(out=kmask[:], in_=kmask_d[:])
                nc.scalar.dma_start(out=trimask[:], in_=trimask_d[:])

            qT = [sb.tile([128, L], BF16, tag=f"qT{m}", name=f"qT{m}")
                  for m in range(2)]
            kT = [sb.tile([128, L], BF16, tag=f"kT{m}", name=f"kT{m}")
                  for m in range(2)]
            vaug = [sb.tile([128, H_PER_CORE, D + 1], BF16, tag=f"va{t}",
                            name=f"va{t}") for t in range(TT)]
            yT = [sb.tile([128, L], BF16, tag=f"yT{m}", name=f"yT{m}")
                  for m in range(2)]
            xT = [sb.tile([128, L], BF16, tag=f"xT{k}", name=f"xTs{k}")
                  for k in range(CT)]
            yR = [sb.tile([128, C], BF16, tag=f"yR{q}", name=f"yR{q}")
                  for q in range(QC)]
            dram = ctx.enter_context(tc.tile_pool(name="dram", bufs=1,
                                                  space="DRAM"))
            rs_ins = [dram.tile([QCHUNK, C], BF16, tag=f"rsi{q}",
                                name=f"rsi{q}") for q in range(QC)]
            rs_outs = [dram.tile([128, C], BF16, tag=f"rso{q}",
                                 name=f"rso{q}") for q in range(QC)]

            def qk_proj(ht):
                for t4 in range(L // 512):
                    sl = bass.ts(t4, 512)
                    for dst, off in ((qT, 0), (kT, DQ)):
                        p = ps.tile([128, 512], F32, tag="psBV", name="psB",
                                    bufs=2)
                        for k in range(CT):
                            nc.tensor.matmul(
                                p[:],
                                wqkv[k][:, off + ht*128:off + (ht+1)*128],
                                xT[k][:, sl],
                                start=(k == 0), stop=(k == CT - 1))
                        nc.vector.tensor_copy(dst[ht][:, sl], p[:])

            def v_proj():
                for t in range(TT):
                    nc.vector.memset(vaug[t][:, :, D:D+1], 1.0)
                    p = ps.tile([128, DQ], F32, tag="psBV", name="psV", bufs=2)
                    for k in range(CT):
                        nc.tensor.matmul(
                            p[:], xT[k][:, t*128:(t+1)*128],
                            wqkv[k][:, 2*DQ:3*DQ],
                            start=(k == 0), stop=(k == CT - 1))
                    nc.vector.tensor_copy(
                        vaug[t][:, :, 0:D],
                        p.rearrange("p (h d) -> p h d", h=H_PER_CORE))

            def attention(ht, qc):
                q0 = qc * QCHUNK
                nkb = (q0 + QCHUNK) // 128
                psys = {}
                for hp in (0, 64):
                    psys[hp] = ps.tile([128, QCHUNK], F32, tag=f"psy{hp}",
                                       name=f"psy{hp}", bufs=1)
                # software-pipelined: score matmuls for key block j+1 are
                # emitted before the P@V matmuls of block j, so the strict-
                # FIFO PE has independent work while ScalarE runs exp(j).
                def emit_s(j, c_lo):
                    # scores only for the causally-live query columns
                    pss = ps.tile([128, 2, QCHUNK], F32, tag="pssP",
                                  name="pssP", bufs=2)
                    for hp in (0, 64):
                        nc.tensor.matmul(
                            pss[:, hp // 64, c_lo:QCHUNK],
                            kT[ht][hp:hp+64, j*128:(j+1)*128],
                            qT[ht][hp:hp+64,
                                   bass.ds(q0 + c_lo, QCHUNK - c_lo)],
                            start=True, stop=True)
                    return pss

                pss_j = emit_s(0, 0)
                for j in range(nkb):
                    c_lo = max(0, j * 128 - q0)
                    pss, pss_j = pss_j, None
                    pt = pt_pool.tile([128, 2, QCHUNK], BF16,
                                      tag="ptP", name="ptP")
                    nc.scalar.activation(
                        pt[:, :, c_lo:QCHUNK], pss[:, :, c_lo:QCHUNK],
                        AF.Exp, bias=kmask[:, j:j+1], scale=0.125)
                    if j + 1 < nkb:
                        pss_j = emit_s(j + 1, max(0, (j + 1) * 128 - q0))
                    if j * 128 >= q0:
                        for hpi in (0, 1):
                            nc.vector.tensor_mul(pt[:, hpi, c_lo:c_lo+128],
                                                 pt[:, hpi, c_lo:c_lo+128],
                                                 trimask[:])
                    for hp in (0, 64):
                        h = 2 * ht + hp // 64
                        nc.tensor.matmul(
                            psys[hp][:65, c_lo:QCHUNK],
                            vaug[j][:, h, :],
                            pt[:, hp // 64, c_lo:QCHUNK],
                            start=(j == 0), stop=(j == nkb - 1))
                for hp in (0, 64):
                    psy = psys[hp]
                    qsl = bass.ds(q0, QCHUNK)
                    rden = den_pool.tile([1, QCHUNK], BF16, tag="rden")
                    with nc.allow_low_precision(
                            reason="1/den in bf16 feeds a bf16 multiply"):
                        nc.vector.reciprocal(rden[:], psy[64:65, :])
                    # broadcast 1/den across 64 partitions with a K=1
                    # matmul into the unused upper rows of the same bank
                    # (keeps the Pool engine free for the collectives);
                    # stage via SBUF: DVE may read only one PSUM operand
                    nc.tensor.matmul(psy[64:128, :], ones1[:], rden[:],
                                     start=True, stop=True)
                    rdb = den_pool.tile([64, QCHUNK], BF16, tag="rdb")
                    with nc.allow_low_precision(
                            reason="bf16 1/den broadcast for bf16 y"):
                        nc.vector.tensor_copy(rdb[:], psy[64:128, :])
                    nc.vector.tensor_mul(yT[ht][hp:hp+64, qsl],
                                         psy[0:64, :], rdb[:])

            def rs_and_proj(qc):
                # partial proj (K = my 256 channels) -> bf16 partial
                # [512, 1024] -> 4-rank ReduceScatter(add) -> my 128 rows.
                q0 = qc * QCHUNK
                rs_in = rs_ins[qc]
                for mt in range(4):
                    qsl = bass.ds(q0 + mt * 128, 128)
                    for half in range(2):
                        p = ps.tile([128, 512], F32, tag="psBV", name="psP",
                                    bufs=2)
                        for m in range(2):
                            nc.tensor.matmul(
                                p[:], yT[m][:, qsl],
                                wp[m][:, half*512:(half+1)*512],
                                start=(m == 0), stop=(m == 1))
                        st = stg.tile([128, 512], BF16, tag="st")
                        nc.vector.tensor_copy(st[:], p[:])
                        nc.sync.dma_start(
                            out=rs_in[mt*128:(mt+1)*128,
                                      half*512:(half+1)*512],
                            in_=st[:])
                if use_collective:
                    nc.gpsimd.collective_compute(
                        "ReduceScatter", mybir.AluOpType.add,
                        ins=[rs_in[:]], outs=[rs_outs[qc][:]],
                        replica_groups=[[0, 1, 2, 3], [4, 5, 6, 7]])
                    # DRAM->SBUF->DRAM double hop: far cheaper than a
                    # DRAM->DRAM DMA and off the SP queue that feeds rs_in
                    nc.scalar.dma_start(out=yR[qc][:], in_=rs_outs[qc][:])
                    nc.scalar.dma_start(out=out_ds[qc][:], in_=yR[qc][:])
                else:
                    nc.sync.dma_start(out=out_ds[qc][:], in_=rs_in[0:128, :])

            for _rep in range(reps):
                load_inputs()
                v_proj()
                qk_proj(0)
                qk_proj(1)
                for qc in range(QC):
                    for ht in range(2):
                        attention(ht, qc)
                    rs_and_proj(qc)

    nc.compile()
    return nc


def host_inputs(x, attn_mask, Wq, Wk, Wv, Wp):
    x = np.asarray(x)
    attn_mask = np.asarray(attn_mask)
    Wq, Wk, Wv, Wp = (np.asarray(a) for a in (Wq, Wk, Wv, Wp))

    def bfc(a):
        return np.ascontiguousarray(
            np.asarray(a, dtype=np.float32)).astype(ml_dtypes.bfloat16)

    tri = np.triu(np.ones((128, 128), np.float32)).astype(ml_dtypes.bfloat16)
    in_maps = []
    for c in range(N_CORES):
        b, g = c // 4, c % 4
        sl = slice(DQ * g, DQ * (g + 1))
        km = np.where(attn_mask[b] != 0, 0.0, NEG).astype(np.float32)
        km = np.ascontiguousarray(km.reshape(NB, 128).T)
        in_maps.append({
            "xT": bfc(x[b].T),
            "wqkvT": bfc(np.concatenate(
                [Wq[sl, :].T, Wk[sl, :].T, Wv[sl, :].T], axis=1)),
            "wpT": bfc(Wp[:, sl].T),
            "kmask": km,
            "trimask": np.ascontiguousarray(tri),
        })
    return in_maps


_CACHED = {}


def kernel(x, attn_mask, Wq, Wk, Wv, Wp):
    if "nc" not in _CACHED:
        _CACHED["nc"] = build_kernel()
    nc = _CACHED["nc"]
    in_maps = host_inputs(x, attn_mask, Wq, Wk, Wv, Wp)
    res = bass_utils.run_bass_kernel_spmd(
        nc, in_maps, core_ids=list(range(N_CORES)))
    out = np.zeros((B, L, C), np.float32)
    for b in range(B):
        # core 4b+g holds rows qc*512 + g*128 + i in its out{qc} tensors
        arr = np.stack(
            [np.stack([np.asarray(res.results[4*b + g][f"out{qc}"],
                                  dtype=np.float32) for qc in range(QC)])
             for g in range(4)], axis=1)
        out[b] = arr.reshape(L, C)
    return out

